# revision 60
# baseline (speedup 1.0000x reference)
"""Sparse cross-attention (squared-ReLU normalizer) on 8 TRN2 NeuronCores.

Sharding: 8 cores = batch(2) x head-group(4). Each core owns one batch and
4 of 16 heads (a 256-wide slice of hsize): Wq/Wkv column-parallel,
Wo row-parallel (bf16 partial outputs summed on host), mask replicated per
batch shard.

Per-core kernel (bf16 matmuls, fp32 PSUM). All DMAs ride the sync-engine
HWDGE queue (packets stripe over all 16 DMA engines; desc-gen on compute
engines would stall them), ordered so A1's inputs land first.

  Stage A: rqT (hs, q) / rkT (hs, s) with weight chunks stationary (reused
  across 4 N=512 streams, 8 live psum accumulation groups); rv
  (s, 4*(64+1)) with kT chunks stationary. 1/sqrt(adim) folded into Wq on
  the host.

  Stage B: one uniform lag-2 software pipeline over steps (half, h, sc)
  with q processed in 1024-halves — AV for step i-2 is emitted under step
  i's scores across head AND half boundaries, so the PE never idles and
  the HAM clock gate stays at 2.4 GHz:
    scores psum [128,1024] = rkT[h,sc]^T @ rqT[h]    (K=64, 2x N=512)
    psum += I_128 @ maskaddT chunk    (additive mask: masked -> -1e30,
      else +nbias, host-prepared; keeps the elementwise chain short)
    r = Relu(psum)  [ACT, bf16]; tT = r*r  [DVE]  (= relu(s')^2)
    av psum [65,1024] += rv[h,sc]^T @ tT   (rv stationary: 64 v cols +
      ones col -> row 64 = denominator)
  Head tail (pipelined 2-4 steps later): den row -> bf16 +eps [ACT],
  partition-broadcast via K=1 ones matmul, reciprocal_approx_fast [DVE],
  oT = av * rec2 [DVE]. Odd heads' oT moves to partitions 64-127 via
  SBUF->SBUF DMA so outproj runs K=128 on head PAIRS:
  out[qc, d] = sum_pr oTpair[pr][:, qc]^T @ wo_pair[pr], bf16 -> DMA.
  Each half's outproj is emitted as a blob a few steps into the next half
  (PE-dense region bridges the boundary).
"""

import numpy as np
import ml_dtypes

BF16 = ml_dtypes.bfloat16

B, Q, S, D = 2, 2048, 2048, 1024
NUM_HEAD, ADIM = 16, 64
HSIZE = NUM_HEAD * ADIM
N_CORES = 8
GROUPS = 4                  # head groups (tensor-parallel dim)
HPG = NUM_HEAD // GROUPS    # 4 heads per core
HS = HPG * ADIM             # 256: per-core hsize slice
IEPS = 1e-32
P = 128

_COMPILED = None


def _build(q=Q, s=S, d=D, hpg=HPG, adim=ADIM):
    """Build + compile the per-core Bass program. Returns the Bacc."""
    from contextlib import ExitStack
    import concourse.bass as bass
    import concourse.mybir as mybir
    import concourse.tile as tile
    from concourse import bacc
    from concourse.masks import make_identity

    fp32 = mybir.dt.float32
    bf16 = mybir.dt.bfloat16

    hs = hpg * adim          # 256
    DC = d // P              # 8 contraction chunks for projections
    SC = s // P              # 16 s chunks
    HC = hs // P             # 2 hsize-slice chunks
    HALF = 1024              # q processed in halves
    NH = q // HALF           # 2
    NW = HALF // 512         # 2 N=512 windows per half
    AV_LAG = 2

    nc = bacc.Bacc("TRN2", target_bir_lowering=False, debug=False,
                   num_devices=N_CORES)

    qT = nc.dram_tensor("qT", [d, q], bf16, kind="ExternalInput").ap()
    kT = nc.dram_tensor("kT", [d, s], bf16, kind="ExternalInput").ap()
    wqT = nc.dram_tensor("wqT", [d, hs], bf16, kind="ExternalInput").ap()
    wkT = nc.dram_tensor("wkT", [d, hs], bf16, kind="ExternalInput").ap()
    wvT = nc.dram_tensor("wvT", [d, hs], bf16, kind="ExternalInput").ap()
    # wo packed in head PAIRS: partitions 0-63 = head 2p, 64-127 = head 2p+1
    wo = nc.dram_tensor("wo", [P, hpg // 2, d], bf16,
                        kind="ExternalInput").ap()
    # additive mask: masked -> -1e30, unmasked -> nbias (host-prepared)
    maskT = nc.dram_tensor("maskT", [s, q], bf16, kind="ExternalInput").ap()
    out = nc.dram_tensor("out", [q, d], bf16, kind="ExternalOutput").ap()

    qT_t = qT.rearrange("(c p) q -> c p q", p=P)        # [DC, 128, q]
    kT_t = kT.rearrange("(c p) s -> c p s", p=P)
    wqT_t = wqT.rearrange("(c p) h -> c p h", p=P)      # [DC, 128, hs]
    wkT_t = wkT.rearrange("(c p) h -> c p h", p=P)
    wvT_t = wvT.rearrange("(c p) h -> c p h", p=P)
    maskT_t = maskT.rearrange("(c p) q -> p c q", p=P)  # [128, SC, q]
    out_t = out.rearrange("(c p) d -> c p d", p=P)      # [q/P, 128, d]

    with tile.TileContext(nc) as tc, ExitStack() as ctx:
        const = ctx.enter_context(tc.tile_pool(name="const", bufs=1))
        wpool = ctx.enter_context(tc.tile_pool(name="w", bufs=1))
        actp = ctx.enter_context(tc.tile_pool(name="act", bufs=1))
        maskp = ctx.enter_context(tc.tile_pool(name="mask", bufs=2))
        rp = ctx.enter_context(tc.tile_pool(name="r", bufs=2))
        tTp = ctx.enter_context(tc.tile_pool(name="tT", bufs=AV_LAG + 1))
        recp = ctx.enter_context(tc.tile_pool(name="rec", bufs=1))
        oTp = ctx.enter_context(tc.tile_pool(name="oT", bufs=hpg))
        outp = ctx.enter_context(tc.tile_pool(name="out", bufs=2))

        # ---- constants ----
        ones64 = const.tile([1, adim], bf16)
        nc.any.memset(ones64[:], 1.0)
        ident = const.tile([P, P], bf16)
        make_identity(nc, ident)

        # ---- resident weights ----
        # One dma_start per tensor: desc-gen (~0.7us/instr) is the scarce
        # resource, a single queue stripes packets over all 16 DMA engines.
        # sync queue carries A1's critical path (wq then x_q chunks);
        # scalar queue carries A2/A3 (wk, wv, x_k); gpsimd carries wo+mask.
        wq_sb = wpool.tile([P, DC, hs], bf16)
        wk_sb = wpool.tile([P, DC, hs], bf16)
        wv_sb = wpool.tile([P, DC, hs], bf16)
        wo_sb = wpool.tile([P, hpg // 2, d], bf16)
        nc.sync.dma_start(wk_sb[:], wkT_t.rearrange("c p h -> p c h"))

        # ---- resident activations ----
        rqT_sb = actp.tile([P, HC, q], bf16)                 # (hs, q)
        rkT_sb = actp.tile([P, HC, s], bf16)                 # (hs, s)
        rv_sb = actp.tile([P, SC, hpg * (adim + 1)], bf16)   # (s, hs + ones)
        nc.any.memset(rv_sb[:], 1.0)        # ones cols survive at 64::65

        mask_sb = [maskp.tile([P, SC, HALF], bf16, tag="mask",
                              name=f"mask{hf}") for hf in range(NH)]

        # ================= stage A =================
        with tc.tile_pool(name="xa", bufs=1) as xpool, \
             tc.tile_pool(name="xq", bufs=DC) as xqpool, \
             tc.tile_pool(name="psA", bufs=1, space="PSUM") as psA:
            # sync queue: (wq, x_q) interleaved per chunk so A1's c-loop
            # starts within a few us and streams just-in-time; then wo.
            # scalar queue: x_k per chunk (A2/A3), wv, then mask half 0.
            # gpsimd SWDGE is NOT used for DMA (~13 GB/s, far too slow).
            x_qs = [xqpool.tile([P, q], bf16, tag="xq", name=f"xq{c}")
                    for c in range(DC)]
            x_k = xpool.tile([P, DC, s], bf16, tag="xk")
            # A1's stream first: (wq, x_q) pairs; x_k (A2, needed ~30us
            # later) behind them; then wv/wo/mask0 (stage A3 / stage B).
            for c in range(DC):
                nc.sync.dma_start(wq_sb[:, c], wqT_t[c])
                if c < 3:
                    # halve the first chunks' transfers: subtile deps let
                    # A1's first matmuls start after 256KB, not 512KB
                    hq = q // 2
                    nc.sync.dma_start(x_qs[c][:, 0:hq], qT_t[c][:, 0:hq])
                    nc.sync.dma_start(x_qs[c][:, hq:], qT_t[c][:, hq:])
                else:
                    nc.sync.dma_start(x_qs[c][:], qT_t[c])
            for c in range(DC):
                nc.sync.dma_start(x_k[:, c], kT_t[c])
            nc.sync.dma_start(wv_sb[:], wvT_t.rearrange("c p h -> p c h"))
            nc.sync.dma_start(wo_sb[:], wo[:])
            for sc in range(SC):
                nc.sync.dma_start(mask_sb[0][:, sc], maskT_t[:, sc, 0:HALF])

            # A1/A2: rqT and rkT. c outer, weight chunks stationary for
            # 4 streams, 8 psum accumulation groups live.
            for w_sb, x_res, dst in ((wq_sb, x_qs, rqT_sb),
                                     (wk_sb, x_k, rkT_sb)):
                ps8 = [psA.tile([P, 512], fp32, tag=f"pa{m * 4 + nq}",
                                name=f"psA_{m}_{nq}")
                       for m in range(HC) for nq in range(q // 512)]
                for c in range(DC):
                    x_c = x_res[c] if isinstance(x_res, list) else x_res[:, c]
                    for m in range(HC):
                        for nq in range(q // 512):
                            nc.tensor.matmul(
                                ps8[m * 4 + nq][:],
                                w_sb[:, c, m * P:(m + 1) * P],
                                x_c[:, nq * 512:(nq + 1) * 512],
                                start=(c == 0), stop=(c == DC - 1))
                for m in range(HC):
                    for nq in range(q // 512):
                        nc.scalar.copy(dst[:, m, nq * 512:(nq + 1) * 512],
                                       ps8[m * 4 + nq][:])

            # A3: rv (kT chunks stationary, wv moving)
            for sc in range(SC):
                ps = psA.tile([P, hs], fp32, tag=f"pa{sc % 2}", name="psA_rv")
                for c in range(DC):
                    nc.tensor.matmul(
                        ps[:], x_k[:, c, sc * P:(sc + 1) * P], wv_sb[:, c],
                        start=(c == 0), stop=(c == DC - 1))
                # scatter heads into 65-strided groups (col 64 stays 1.0)
                nc.scalar.copy(
                    rv_sb[:, sc].rearrange("p (h c) -> p h c",
                                           c=adim + 1)[:, :, 0:adim],
                    ps[:].rearrange("p (h c) -> p h c", c=adim))

        # ================= stage B =================
        with tc.tile_pool(name="psB", bufs=1, space="PSUM") as psB:
            tTs = {}
            av_ps = {}
            oT = {}

            def scores_step(half, h, sc):
                qlo = half * HALF
                hc, hp = h // 2, (h % 2) * adim
                ps = psB.tile([P, HALF], fp32, tag="sc", bufs=2,
                              name="ps_sc")
                for w in range(NW):
                    nc.tensor.matmul(
                        ps[:, w * 512:(w + 1) * 512],
                        rkT_sb[hp:hp + adim, hc, sc * P:(sc + 1) * P],
                        rqT_sb[hp:hp + adim, hc,
                               qlo + w * 512:qlo + (w + 1) * 512],
                        start=True, stop=False)
                # accumulate additive mask (+nbias) via identity matmul
                for w in range(NW):
                    nc.tensor.matmul(
                        ps[:, w * 512:(w + 1) * 512], ident[:],
                        mask_sb[half][:, sc, w * 512:(w + 1) * 512],
                        start=False, stop=True)
                # r = relu(scores + maskadd); t = r^2 (= relu(s')*s')
                r = rp.tile([P, HALF], bf16, tag="r", name="r_t")
                nc.scalar.activation(r[:], ps[:],
                                     mybir.ActivationFunctionType.Relu)
                tT = tTp.tile([P, HALF], bf16, tag="tT", name="tT_t")
                nc.vector.tensor_mul(tT[:], r[:], r[:])
                tTs[(half, h, sc)] = tT

            def av_step(half, h, sc):
                if sc == 0:
                    av_ps[h] = psB.tile([adim + 1, HALF], fp32, tag="av",
                                        bufs=2, name=f"av{h}")
                tT = tTs.pop((half, h, sc))
                for w in range(NW):
                    nc.tensor.matmul(
                        av_ps[h][:, w * 512:(w + 1) * 512],
                        rv_sb[:, sc, h * (adim + 1):(h + 1) * (adim + 1)],
                        tT[:, w * 512:(w + 1) * 512],
                        start=(sc == 0), stop=(sc == SC - 1))

            def scale_head_a(h):
                # den row (av psum row 64) -> bf16 SBUF, +eps guard
                av = av_ps[h]
                den = recp.tile([1, HALF], bf16, tag="den", name="den_t")
                nc.scalar.activation(den[:], av[adim:adim + 1, :],
                                     mybir.ActivationFunctionType.Copy,
                                     bias=1e-20)
                return den

            def scale_head_b(h, den):
                # broadcast den across 64 partitions via K=1 matmul,
                # then fast approximate reciprocal into SBUF fp32
                den2 = psB.tile([adim, HALF], fp32, tag="sc", bufs=2,
                                name="den2_t")
                for w in range(NW):
                    nc.tensor.matmul(den2[:, w * 512:(w + 1) * 512],
                                     ones64[:],
                                     den[:, w * 512:(w + 1) * 512],
                                     start=True, stop=True)
                rec2 = recp.tile([adim, HALF], fp32, tag="rec2",
                                 bufs=1, name="rec2_sb")
                nc.vector.reciprocal_approx_fast(rec2[:], den2[:])
                return rec2

            def scale_head_c(half, h, rec2):
                # even heads write pair partitions 0-63 directly; odd heads
                # go via a temp tile + SBUF->SBUF DMA into partitions 64-127
                # (compute engines cannot cross partitions, DMA can)
                av = av_ps.pop(h)
                pr = h // 2
                if h % 2 == 0:
                    o = oTp.tile([P, HALF], bf16, tag="oT",
                                 name=f"oT{half}_{pr}")
                    oT[(half, pr)] = o
                    nc.vector.tensor_mul(o[0:adim, :], av[0:adim, :], rec2[:])
                else:
                    tmp = oTp.tile([adim, HALF], bf16, tag="oTtmp", bufs=2,
                                   name="oTtmp")
                    nc.vector.tensor_mul(tmp[:], av[0:adim, :], rec2[:])
                    nc.sync.dma_start(oT[(half, pr)][adim:P, :], tmp[:])

            def outproj_qc(half, qc):
                # K=128: heads accumulated in pairs
                ps2 = psB.tile([P, d], fp32, tag="sc", bufs=2,
                               name=f"op{qc}")
                for pr in range(hpg // 2):
                    for dc in range(d // 512):
                        nc.tensor.matmul(
                            ps2[:, dc * 512:(dc + 1) * 512],
                            oT[(half, pr)][:, qc * P:(qc + 1) * P],
                            wo_sb[:, pr, dc * 512:(dc + 1) * 512],
                            start=(pr == 0), stop=(pr == hpg // 2 - 1))
                if qc == HALF // P - 1:
                    for pr in range(hpg // 2):
                        oT.pop((half, pr))
                ob = outp.tile([P, d], bf16, tag="ob", name="ob_t")
                nc.scalar.copy(ob[:], ps2[:])
                nc.sync.dma_start(out_t[half * (HALF // P) + qc], ob[:])

            # Uniform lag-2 software pipeline across head AND half
            # boundaries: scores at step i, AV at step i-2 — the PE stream
            # never pauses for a head's tail (that av flush happens under
            # the next head's first scores), keeping the HAM gate at 2.4GHz.
            # den/scale for head X runs 2-4 steps after its last AV; the
            # previous half's outproj runs as a blob early in the next half.
            STEPS = [(half, h, sc) for half in range(NH)
                     for h in range(hpg) for sc in range(SC)]
            N = len(STEPS)
            pend_a = {}   # step idx -> (half, h) whose den copy to emit
            pend_b = {}   # step idx -> (half, h, den) for broadcast+scale
            pend_op = {}  # step idx -> half whose outproj to emit
            for i in range(N + 2 * AV_LAG + 2):
                if i < N:
                    half, h, sc = STEPS[i]
                    if h == 0 and sc == 0 and half + 1 < NH:
                        for msc in range(SC):
                            nc.sync.dma_start(
                                mask_sb[half + 1][:, msc],
                                maskT_t[:, msc,
                                        (half + 1) * HALF:(half + 2) * HALF])
                    scores_step(half, h, sc)
                j = i - AV_LAG
                if 0 <= j < N:
                    jhalf, jh, jsc = STEPS[j]
                    av_step(jhalf, jh, jsc)
                    if jsc == SC - 1:
                        pend_a[i + 1] = (jhalf, jh)
                if i in pend_a:
                    ahalf, ah = pend_a.pop(i)
                    den = scale_head_a(ah)
                    pend_b[i + 2] = (ahalf, ah, den)
                if i in pend_b:
                    bhalf, bh, den = pend_b.pop(i)
                    rec2 = scale_head_b(bh, den)
                    scale_head_c(bhalf, bh, rec2)
                    if bh == hpg - 1:
                        pend_op[i + 1] = bhalf
                if i in pend_op:
                    for qc in range(HALF // P):
                        outproj_qc(pend_op[i], qc)
                    del pend_op[i]

    nc.compile()
    return nc


def _shard_inputs(iQ, iK, mask, Wq, Wkv, Wo, nbias):
    in_maps = []
    nb = np.float32(np.asarray(nbias).reshape(-1)[0])
    # additive mask: masked -> -1e30 (relu zeroes it), unmasked -> +nbias
    maskT_by_b = [np.ascontiguousarray(
        np.where(mask[b].T, np.float32(-1e30), nb)).astype(BF16)
        for b in range(B)]
    qT_by_b = [np.ascontiguousarray(iQ[b].T).astype(BF16) for b in range(B)]
    kT_by_b = [np.ascontiguousarray(iK[b].T).astype(BF16) for b in range(B)]
    scale = np.float32(1.0 / np.sqrt(np.float32(ADIM)))
    for ci in range(N_CORES):
        b, g = ci // GROUPS, ci % GROUPS
        hsl = slice(g * HS, (g + 1) * HS)
        # wo: [128, hpg//2, d]; pair p = heads (2p, 2p+1) stacked on the
        # partition dim
        wo_np = np.stack(
            [np.concatenate(
                [Wo[:, g * HS + h * ADIM:g * HS + (h + 1) * ADIM].T
                 for h in (2 * p, 2 * p + 1)], axis=0)
             for p in range(HPG // 2)], axis=1).astype(BF16)
        wo_np = np.ascontiguousarray(wo_np)
        in_maps.append({
            "qT": qT_by_b[b],
            "kT": kT_by_b[b],
            "wqT": np.ascontiguousarray((Wq[hsl] * scale).T).astype(BF16),
            "wkT": np.ascontiguousarray(Wkv[hsl].T).astype(BF16),
            "wvT": np.ascontiguousarray(
                Wkv[HSIZE + g * HS:HSIZE + (g + 1) * HS].T).astype(BF16),
            "wo": wo_np,
            "maskT": maskT_by_b[b],
        })
    return in_maps


def kernel(iQ, iK, mask, Wq, Wkv, Wo, nbias):
    global _COMPILED
    from concourse.bass_utils import run_bass_kernel_spmd

    if _COMPILED is None:
        _COMPILED = _build()
    in_maps = _shard_inputs(np.asarray(iQ, np.float32), np.asarray(iK, np.float32),
                            np.asarray(mask), np.asarray(Wq, np.float32),
                            np.asarray(Wkv, np.float32), np.asarray(Wo, np.float32),
                            np.asarray(nbias, np.float32))
    res = run_bass_kernel_spmd(_COMPILED, in_maps, list(range(N_CORES))).results
    out = np.zeros((B, Q, D), np.float32)
    for ci in range(N_CORES):
        out[ci // GROUPS] += np.asarray(res[ci]["out"], np.float32)
    return out


# revision 61
# speedup vs baseline: 1.0100x; 1.0100x over previous
"""Sparse cross-attention (squared-ReLU normalizer) on 8 TRN2 NeuronCores.

Sharding: 8 cores = batch(2) x head-group(4). Each core owns one batch and
4 of 16 heads (a 256-wide slice of hsize): Wq/Wkv column-parallel,
Wo row-parallel (bf16 partial outputs summed on host), mask replicated per
batch shard.

Per-core kernel (bf16 matmuls, fp32 PSUM). All DMAs ride the sync-engine
HWDGE queue (packets stripe over all 16 DMA engines; desc-gen on compute
engines would stall them), ordered so A1's inputs land first.

  Stage A: rqT (hs, q) / rkT (hs, s) with weight chunks stationary (reused
  across 4 N=512 streams, 8 live psum accumulation groups); rv
  (s, 4*(64+1)) with kT chunks stationary. 1/sqrt(adim) folded into Wq on
  the host.

  Stage B: one uniform lag-2 software pipeline over steps (half, h, sc)
  with q processed in 1024-halves — AV for step i-2 is emitted under step
  i's scores across head AND half boundaries, so the PE never idles and
  the HAM clock gate stays at 2.4 GHz:
    scores psum [128,1024] = rkT[h,sc]^T @ rqT[h]    (K=64, 2x N=512)
    psum += I_128 @ maskaddT chunk    (additive mask: masked -> -1e30,
      else +nbias, host-prepared; keeps the elementwise chain short)
    r = Relu(psum)  [ACT, bf16]; tT = r*r  [DVE]  (= relu(s')^2)
    av psum [65,1024] += rv[h,sc]^T @ tT   (rv stationary: 64 v cols +
      ones col -> row 64 = denominator)
  Head tail (pipelined 2-4 steps later): den row -> bf16 +eps [ACT],
  partition-broadcast via K=1 ones matmul, reciprocal_approx_fast [DVE],
  oT = av * rec2 [DVE]. Odd heads' oT moves to partitions 64-127 via
  SBUF->SBUF DMA so outproj runs K=128 on head PAIRS:
  out[qc, d] = sum_pr oTpair[pr][:, qc]^T @ wo_pair[pr], bf16 -> DMA.
  Each half's outproj is emitted as a blob a few steps into the next half
  (PE-dense region bridges the boundary).
"""

import numpy as np
import ml_dtypes

BF16 = ml_dtypes.bfloat16

B, Q, S, D = 2, 2048, 2048, 1024
NUM_HEAD, ADIM = 16, 64
HSIZE = NUM_HEAD * ADIM
N_CORES = 8
GROUPS = 4                  # head groups (tensor-parallel dim)
HPG = NUM_HEAD // GROUPS    # 4 heads per core
HS = HPG * ADIM             # 256: per-core hsize slice
IEPS = 1e-32
P = 128

_COMPILED = None


def _build(q=Q, s=S, d=D, hpg=HPG, adim=ADIM):
    """Build + compile the per-core Bass program. Returns the Bacc."""
    from contextlib import ExitStack
    import concourse.bass as bass
    import concourse.mybir as mybir
    import concourse.tile as tile
    from concourse import bacc
    from concourse.masks import make_identity

    fp32 = mybir.dt.float32
    bf16 = mybir.dt.bfloat16

    hs = hpg * adim          # 256
    DC = d // P              # 8 contraction chunks for projections
    SC = s // P              # 16 s chunks
    HC = hs // P             # 2 hsize-slice chunks
    HALF = 1024              # q processed in halves
    NH = q // HALF           # 2
    NW = HALF // 512         # 2 N=512 windows per half
    AV_LAG = 2

    nc = bacc.Bacc("TRN2", target_bir_lowering=False, debug=False,
                   num_devices=N_CORES)

    qT = nc.dram_tensor("qT", [d, q], bf16, kind="ExternalInput").ap()
    kT = nc.dram_tensor("kT", [d, s], bf16, kind="ExternalInput").ap()
    wqT = nc.dram_tensor("wqT", [d, hs], bf16, kind="ExternalInput").ap()
    wkT = nc.dram_tensor("wkT", [d, hs], bf16, kind="ExternalInput").ap()
    wvT = nc.dram_tensor("wvT", [d, hs], bf16, kind="ExternalInput").ap()
    # wo packed in head PAIRS: partitions 0-63 = head 2p, 64-127 = head 2p+1
    wo = nc.dram_tensor("wo", [P, hpg // 2, d], bf16,
                        kind="ExternalInput").ap()
    # additive mask: masked -> -1e30, unmasked -> nbias (host-prepared)
    maskT = nc.dram_tensor("maskT", [s, q], bf16, kind="ExternalInput").ap()
    out = nc.dram_tensor("out", [q, d], bf16, kind="ExternalOutput").ap()

    qT_t = qT.rearrange("(c p) q -> c p q", p=P)        # [DC, 128, q]
    kT_t = kT.rearrange("(c p) s -> c p s", p=P)
    wqT_t = wqT.rearrange("(c p) h -> c p h", p=P)      # [DC, 128, hs]
    wkT_t = wkT.rearrange("(c p) h -> c p h", p=P)
    wvT_t = wvT.rearrange("(c p) h -> c p h", p=P)
    maskT_t = maskT.rearrange("(c p) q -> p c q", p=P)  # [128, SC, q]
    out_t = out.rearrange("(c p) d -> c p d", p=P)      # [q/P, 128, d]

    with tile.TileContext(nc) as tc, ExitStack() as ctx:
        const = ctx.enter_context(tc.tile_pool(name="const", bufs=1))
        wpool = ctx.enter_context(tc.tile_pool(name="w", bufs=1))
        actp = ctx.enter_context(tc.tile_pool(name="act", bufs=1))
        maskp = ctx.enter_context(tc.tile_pool(name="mask", bufs=2))
        rp = ctx.enter_context(tc.tile_pool(name="r", bufs=2))
        tTp = ctx.enter_context(tc.tile_pool(name="tT", bufs=AV_LAG + 1))
        recp = ctx.enter_context(tc.tile_pool(name="rec", bufs=1))
        oTp = ctx.enter_context(tc.tile_pool(name="oT", bufs=hpg))
        outp = ctx.enter_context(tc.tile_pool(name="out", bufs=2))

        # ---- constants ----
        ones64 = const.tile([1, adim], bf16)
        nc.any.memset(ones64[:], 1.0)
        ident = const.tile([P, P], bf16)
        make_identity(nc, ident)

        # ---- resident weights ----
        # One dma_start per tensor: desc-gen (~0.7us/instr) is the scarce
        # resource, a single queue stripes packets over all 16 DMA engines.
        # sync queue carries A1's critical path (wq then x_q chunks);
        # scalar queue carries A2/A3 (wk, wv, x_k); gpsimd carries wo+mask.
        wq_sb = wpool.tile([P, DC, hs], bf16)
        wk_sb = wpool.tile([P, DC, hs], bf16)
        wv_sb = wpool.tile([P, DC, hs], bf16)
        wo_sb = wpool.tile([P, hpg // 2, d], bf16)
        nc.sync.dma_start(wk_sb[:], wkT_t.rearrange("c p h -> p c h"))

        # ---- resident activations ----
        rqT_sb = actp.tile([P, HC, q], bf16)                 # (hs, q)
        rkT_sb = actp.tile([P, HC, s], bf16)                 # (hs, s)
        rv_sb = actp.tile([P, SC, hpg * (adim + 1)], bf16)   # (s, hs + ones)
        nc.any.memset(rv_sb[:], 1.0)        # ones cols survive at 64::65

        mask_sb = [maskp.tile([P, SC, HALF], bf16, tag="mask",
                              name=f"mask{hf}") for hf in range(NH)]

        # ================= stage A =================
        with tc.tile_pool(name="xa", bufs=1) as xpool, \
             tc.tile_pool(name="xq", bufs=DC) as xqpool, \
             tc.tile_pool(name="psA", bufs=1, space="PSUM") as psA:
            # sync queue: (wq, x_q) interleaved per chunk so A1's c-loop
            # starts within a few us and streams just-in-time; then wo.
            # scalar queue: x_k per chunk (A2/A3), wv, then mask half 0.
            # gpsimd SWDGE is NOT used for DMA (~13 GB/s, far too slow).
            x_qs = [xqpool.tile([P, q], bf16, tag="xq", name=f"xq{c}")
                    for c in range(DC)]
            x_k = xpool.tile([P, DC, s], bf16, tag="xk")
            # A1's stream first: (wq, x_q) pairs; x_k (A2, needed ~30us
            # later) behind them; then wv/wo/mask0 (stage A3 / stage B).
            for c in range(DC):
                nc.sync.dma_start(wq_sb[:, c], wqT_t[c])
                nc.sync.dma_start(x_qs[c][:], qT_t[c])
            for c in range(DC):
                nc.sync.dma_start(x_k[:, c], kT_t[c])
            nc.sync.dma_start(wv_sb[:], wvT_t.rearrange("c p h -> p c h"))
            nc.sync.dma_start(wo_sb[:], wo[:])
            for sc in range(SC):
                nc.sync.dma_start(mask_sb[0][:, sc], maskT_t[:, sc, 0:HALF])

            # A1/A2: rqT and rkT. c outer, weight chunks stationary for
            # 4 streams, 8 psum accumulation groups live.
            for w_sb, x_res, dst in ((wq_sb, x_qs, rqT_sb),
                                     (wk_sb, x_k, rkT_sb)):
                ps8 = [psA.tile([P, 512], fp32, tag=f"pa{m * 4 + nq}",
                                name=f"psA_{m}_{nq}")
                       for m in range(HC) for nq in range(q // 512)]
                for c in range(DC):
                    x_c = x_res[c] if isinstance(x_res, list) else x_res[:, c]
                    for m in range(HC):
                        for nq in range(q // 512):
                            nc.tensor.matmul(
                                ps8[m * 4 + nq][:],
                                w_sb[:, c, m * P:(m + 1) * P],
                                x_c[:, nq * 512:(nq + 1) * 512],
                                start=(c == 0), stop=(c == DC - 1))
                for m in range(HC):
                    for nq in range(q // 512):
                        nc.scalar.copy(dst[:, m, nq * 512:(nq + 1) * 512],
                                       ps8[m * 4 + nq][:])

            # A3: rv (kT chunks stationary, wv moving)
            for sc in range(SC):
                ps = psA.tile([P, hs], fp32, tag=f"pa{sc % 2}", name="psA_rv")
                for c in range(DC):
                    nc.tensor.matmul(
                        ps[:], x_k[:, c, sc * P:(sc + 1) * P], wv_sb[:, c],
                        start=(c == 0), stop=(c == DC - 1))
                # scatter heads into 65-strided groups (col 64 stays 1.0)
                nc.scalar.copy(
                    rv_sb[:, sc].rearrange("p (h c) -> p h c",
                                           c=adim + 1)[:, :, 0:adim],
                    ps[:].rearrange("p (h c) -> p h c", c=adim))

        # ================= stage B =================
        with tc.tile_pool(name="psB", bufs=1, space="PSUM") as psB:
            tTs = {}
            av_ps = {}
            oT = {}

            def scores_step(half, h, sc):
                qlo = half * HALF
                hc, hp = h // 2, (h % 2) * adim
                ps = psB.tile([P, HALF], fp32, tag="sc", bufs=2,
                              name="ps_sc")
                for w in range(NW):
                    nc.tensor.matmul(
                        ps[:, w * 512:(w + 1) * 512],
                        rkT_sb[hp:hp + adim, hc, sc * P:(sc + 1) * P],
                        rqT_sb[hp:hp + adim, hc,
                               qlo + w * 512:qlo + (w + 1) * 512],
                        start=True, stop=False)
                # accumulate additive mask (+nbias) via identity matmul
                for w in range(NW):
                    nc.tensor.matmul(
                        ps[:, w * 512:(w + 1) * 512], ident[:],
                        mask_sb[half][:, sc, w * 512:(w + 1) * 512],
                        start=False, stop=True)
                # r = relu(scores + maskadd); t = r^2 (= relu(s')*s')
                r = rp.tile([P, HALF], bf16, tag="r", name="r_t")
                nc.scalar.activation(r[:], ps[:],
                                     mybir.ActivationFunctionType.Relu)
                tT = tTp.tile([P, HALF], bf16, tag="tT", name="tT_t")
                nc.vector.tensor_mul(tT[:], r[:], r[:])
                tTs[(half, h, sc)] = tT

            def av_step(half, h, sc):
                if sc == 0:
                    av_ps[h] = psB.tile([adim + 1, HALF], fp32, tag="av",
                                        bufs=2, name=f"av{h}")
                tT = tTs.pop((half, h, sc))
                for w in range(NW):
                    nc.tensor.matmul(
                        av_ps[h][:, w * 512:(w + 1) * 512],
                        rv_sb[:, sc, h * (adim + 1):(h + 1) * (adim + 1)],
                        tT[:, w * 512:(w + 1) * 512],
                        start=(sc == 0), stop=(sc == SC - 1))

            def scale_head_a(h):
                # den row (av psum row 64) -> bf16 SBUF, +eps guard
                av = av_ps[h]
                den = recp.tile([1, HALF], bf16, tag="den", name="den_t")
                nc.scalar.activation(den[:], av[adim:adim + 1, :],
                                     mybir.ActivationFunctionType.Copy,
                                     bias=1e-20)
                return den

            def scale_head_b(h, den):
                # broadcast den across 64 partitions via K=1 matmul,
                # then fast approximate reciprocal into SBUF fp32
                den2 = psB.tile([adim, HALF], fp32, tag="sc", bufs=2,
                                name="den2_t")
                for w in range(NW):
                    nc.tensor.matmul(den2[:, w * 512:(w + 1) * 512],
                                     ones64[:],
                                     den[:, w * 512:(w + 1) * 512],
                                     start=True, stop=True)
                rec2 = recp.tile([adim, HALF], fp32, tag="rec2",
                                 bufs=1, name="rec2_sb")
                nc.vector.reciprocal_approx_fast(rec2[:], den2[:])
                return rec2

            def scale_head_c(half, h, rec2):
                # even heads write pair partitions 0-63 directly; odd heads
                # go via a temp tile + SBUF->SBUF DMA into partitions 64-127
                # (compute engines cannot cross partitions, DMA can)
                av = av_ps.pop(h)
                pr = h // 2
                if h % 2 == 0:
                    o = oTp.tile([P, HALF], bf16, tag="oT",
                                 name=f"oT{half}_{pr}")
                    oT[(half, pr)] = o
                    nc.vector.tensor_mul(o[0:adim, :], av[0:adim, :], rec2[:])
                else:
                    tmp = oTp.tile([adim, HALF], bf16, tag="oTtmp", bufs=2,
                                   name="oTtmp")
                    nc.vector.tensor_mul(tmp[:], av[0:adim, :], rec2[:])
                    nc.sync.dma_start(oT[(half, pr)][adim:P, :], tmp[:])

            def outproj_qc(half, qc):
                # K=128: heads accumulated in pairs
                ps2 = psB.tile([P, d], fp32, tag="sc", bufs=2,
                               name=f"op{qc}")
                for pr in range(hpg // 2):
                    for dc in range(d // 512):
                        nc.tensor.matmul(
                            ps2[:, dc * 512:(dc + 1) * 512],
                            oT[(half, pr)][:, qc * P:(qc + 1) * P],
                            wo_sb[:, pr, dc * 512:(dc + 1) * 512],
                            start=(pr == 0), stop=(pr == hpg // 2 - 1))
                if qc == HALF // P - 1:
                    for pr in range(hpg // 2):
                        oT.pop((half, pr))
                ob = outp.tile([P, d], bf16, tag="ob", name="ob_t")
                nc.scalar.copy(ob[:], ps2[:])
                nc.sync.dma_start(out_t[half * (HALF // P) + qc], ob[:])

            # Uniform lag-2 software pipeline across head AND half
            # boundaries: scores at step i, AV at step i-2 — the PE stream
            # never pauses for a head's tail (that av flush happens under
            # the next head's first scores), keeping the HAM gate at 2.4GHz.
            # den/scale for head X runs 2-4 steps after its last AV; the
            # previous half's outproj runs as a blob early in the next half.
            STEPS = [(half, h, sc) for half in range(NH)
                     for h in range(hpg) for sc in range(SC)]
            N = len(STEPS)
            pend_a = {}   # step idx -> (half, h) whose den copy to emit
            pend_b = {}   # step idx -> (half, h, den) for broadcast+scale
            pend_op = {}  # step idx -> half whose outproj to emit
            for i in range(N + 2 * AV_LAG + 2):
                if i < N:
                    half, h, sc = STEPS[i]
                    if h == 0 and sc == 0 and half + 1 < NH:
                        for msc in range(SC):
                            nc.sync.dma_start(
                                mask_sb[half + 1][:, msc],
                                maskT_t[:, msc,
                                        (half + 1) * HALF:(half + 2) * HALF])
                    scores_step(half, h, sc)
                j = i - AV_LAG
                if 0 <= j < N:
                    jhalf, jh, jsc = STEPS[j]
                    av_step(jhalf, jh, jsc)
                    if jsc == SC - 1:
                        pend_a[i + 1] = (jhalf, jh)
                if i in pend_a:
                    ahalf, ah = pend_a.pop(i)
                    den = scale_head_a(ah)
                    pend_b[i + 2] = (ahalf, ah, den)
                if i in pend_b:
                    bhalf, bh, den = pend_b.pop(i)
                    rec2 = scale_head_b(bh, den)
                    scale_head_c(bhalf, bh, rec2)
                    if bh == hpg - 1:
                        pend_op[i + 1] = bhalf
                if i in pend_op:
                    for qc in range(HALF // P):
                        outproj_qc(pend_op[i], qc)
                    del pend_op[i]

    nc.compile()
    return nc


def _shard_inputs(iQ, iK, mask, Wq, Wkv, Wo, nbias):
    in_maps = []
    nb = np.float32(np.asarray(nbias).reshape(-1)[0])
    # additive mask: masked -> -1e30 (relu zeroes it), unmasked -> +nbias
    maskT_by_b = [np.ascontiguousarray(
        np.where(mask[b].T, np.float32(-1e30), nb)).astype(BF16)
        for b in range(B)]
    qT_by_b = [np.ascontiguousarray(iQ[b].T).astype(BF16) for b in range(B)]
    kT_by_b = [np.ascontiguousarray(iK[b].T).astype(BF16) for b in range(B)]
    scale = np.float32(1.0 / np.sqrt(np.float32(ADIM)))
    for ci in range(N_CORES):
        b, g = ci // GROUPS, ci % GROUPS
        hsl = slice(g * HS, (g + 1) * HS)
        # wo: [128, hpg//2, d]; pair p = heads (2p, 2p+1) stacked on the
        # partition dim
        wo_np = np.stack(
            [np.concatenate(
                [Wo[:, g * HS + h * ADIM:g * HS + (h + 1) * ADIM].T
                 for h in (2 * p, 2 * p + 1)], axis=0)
             for p in range(HPG // 2)], axis=1).astype(BF16)
        wo_np = np.ascontiguousarray(wo_np)
        in_maps.append({
            "qT": qT_by_b[b],
            "kT": kT_by_b[b],
            "wqT": np.ascontiguousarray((Wq[hsl] * scale).T).astype(BF16),
            "wkT": np.ascontiguousarray(Wkv[hsl].T).astype(BF16),
            "wvT": np.ascontiguousarray(
                Wkv[HSIZE + g * HS:HSIZE + (g + 1) * HS].T).astype(BF16),
            "wo": wo_np,
            "maskT": maskT_by_b[b],
        })
    return in_maps


def kernel(iQ, iK, mask, Wq, Wkv, Wo, nbias):
    global _COMPILED
    from concourse.bass_utils import run_bass_kernel_spmd

    if _COMPILED is None:
        _COMPILED = _build()
    in_maps = _shard_inputs(np.asarray(iQ, np.float32), np.asarray(iK, np.float32),
                            np.asarray(mask), np.asarray(Wq, np.float32),
                            np.asarray(Wkv, np.float32), np.asarray(Wo, np.float32),
                            np.asarray(nbias, np.float32))
    res = run_bass_kernel_spmd(_COMPILED, in_maps, list(range(N_CORES))).results
    out = np.zeros((B, Q, D), np.float32)
    for ci in range(N_CORES):
        out[ci // GROUPS] += np.asarray(res[ci]["out"], np.float32)
    return out


# revision 63
# speedup vs baseline: 1.0137x; 1.0037x over previous
"""Sparse cross-attention (squared-ReLU normalizer) on 8 TRN2 NeuronCores.

Sharding: 8 cores = batch(2) x head-group(4). Each core owns one batch and
4 of 16 heads (a 256-wide slice of hsize): Wq/Wkv column-parallel,
Wo row-parallel (bf16 partial outputs summed on host), mask replicated per
batch shard.

Per-core kernel (bf16 matmuls, fp32 PSUM). All DMAs ride the sync-engine
HWDGE queue (packets stripe over all 16 DMA engines; desc-gen on compute
engines would stall them), ordered so A1's inputs land first.

  Stage A: rqT (hs, q) / rkT (hs, s) with weight chunks stationary (reused
  across 4 N=512 streams, 8 live psum accumulation groups); rv
  (s, 4*(64+1)) with kT chunks stationary. 1/sqrt(adim) folded into Wq on
  the host.

  Stage B: one uniform lag-2 software pipeline over steps (half, h, sc)
  with q processed in 1024-halves — AV for step i-2 is emitted under step
  i's scores across head AND half boundaries, so the PE never idles and
  the HAM clock gate stays at 2.4 GHz:
    scores psum [128,1024] = rkT[h,sc]^T @ rqT[h]    (K=64, 2x N=512)
    psum += I_128 @ maskaddT chunk    (additive mask: masked -> -1e30,
      else +nbias, host-prepared; keeps the elementwise chain short)
    r = Relu(psum)  [ACT, bf16]; tT = r*r  [DVE]  (= relu(s')^2)
    av psum [65,1024] += rv[h,sc]^T @ tT   (rv stationary: 64 v cols +
      ones col -> row 64 = denominator)
  Head tail (pipelined 2-4 steps later): den row -> bf16 +eps [ACT],
  partition-broadcast via K=1 ones matmul, reciprocal_approx_fast [DVE],
  oT = av * rec2 [DVE]. Odd heads' oT moves to partitions 64-127 via
  SBUF->SBUF DMA so outproj runs K=128 on head PAIRS:
  out[qc, d] = sum_pr oTpair[pr][:, qc]^T @ wo_pair[pr], bf16 -> DMA.
  Each half's outproj is emitted as a blob a few steps into the next half
  (PE-dense region bridges the boundary).
"""

import numpy as np
import ml_dtypes

BF16 = ml_dtypes.bfloat16

B, Q, S, D = 2, 2048, 2048, 1024
NUM_HEAD, ADIM = 16, 64
HSIZE = NUM_HEAD * ADIM
N_CORES = 8
GROUPS = 4                  # head groups (tensor-parallel dim)
HPG = NUM_HEAD // GROUPS    # 4 heads per core
HS = HPG * ADIM             # 256: per-core hsize slice
IEPS = 1e-32
P = 128

_COMPILED = None


def _build(q=Q, s=S, d=D, hpg=HPG, adim=ADIM):
    """Build + compile the per-core Bass program. Returns the Bacc."""
    from contextlib import ExitStack
    import concourse.bass as bass
    import concourse.mybir as mybir
    import concourse.tile as tile
    from concourse import bacc
    from concourse.masks import make_identity

    fp32 = mybir.dt.float32
    bf16 = mybir.dt.bfloat16

    hs = hpg * adim          # 256
    DC = d // P              # 8 contraction chunks for projections
    SC = s // P              # 16 s chunks
    HC = hs // P             # 2 hsize-slice chunks
    HALF = 1024              # q processed in halves
    NH = q // HALF           # 2
    NW = HALF // 512         # 2 N=512 windows per half
    AV_LAG = 3

    nc = bacc.Bacc("TRN2", target_bir_lowering=False, debug=False,
                   num_devices=N_CORES)

    qT = nc.dram_tensor("qT", [d, q], bf16, kind="ExternalInput").ap()
    kT = nc.dram_tensor("kT", [d, s], bf16, kind="ExternalInput").ap()
    wqT = nc.dram_tensor("wqT", [d, hs], bf16, kind="ExternalInput").ap()
    wkT = nc.dram_tensor("wkT", [d, hs], bf16, kind="ExternalInput").ap()
    wvT = nc.dram_tensor("wvT", [d, hs], bf16, kind="ExternalInput").ap()
    # wo packed in head PAIRS: partitions 0-63 = head 2p, 64-127 = head 2p+1
    wo = nc.dram_tensor("wo", [P, hpg // 2, d], bf16,
                        kind="ExternalInput").ap()
    # additive mask: masked -> -1e30, unmasked -> nbias (host-prepared)
    maskT = nc.dram_tensor("maskT", [s, q], bf16, kind="ExternalInput").ap()
    out = nc.dram_tensor("out", [q, d], bf16, kind="ExternalOutput").ap()

    qT_t = qT.rearrange("(c p) q -> c p q", p=P)        # [DC, 128, q]
    kT_t = kT.rearrange("(c p) s -> c p s", p=P)
    wqT_t = wqT.rearrange("(c p) h -> c p h", p=P)      # [DC, 128, hs]
    wkT_t = wkT.rearrange("(c p) h -> c p h", p=P)
    wvT_t = wvT.rearrange("(c p) h -> c p h", p=P)
    maskT_t = maskT.rearrange("(c p) q -> p c q", p=P)  # [128, SC, q]
    out_t = out.rearrange("(c p) d -> c p d", p=P)      # [q/P, 128, d]

    with tile.TileContext(nc) as tc, ExitStack() as ctx:
        const = ctx.enter_context(tc.tile_pool(name="const", bufs=1))
        wpool = ctx.enter_context(tc.tile_pool(name="w", bufs=1))
        actp = ctx.enter_context(tc.tile_pool(name="act", bufs=1))
        maskp = ctx.enter_context(tc.tile_pool(name="mask", bufs=2))
        rp = ctx.enter_context(tc.tile_pool(name="r", bufs=2))
        tTp = ctx.enter_context(tc.tile_pool(name="tT", bufs=AV_LAG + 1))
        recp = ctx.enter_context(tc.tile_pool(name="rec", bufs=1))
        oTp = ctx.enter_context(tc.tile_pool(name="oT", bufs=hpg))
        outp = ctx.enter_context(tc.tile_pool(name="out", bufs=2))

        # ---- constants ----
        ones64 = const.tile([1, adim], bf16)
        nc.any.memset(ones64[:], 1.0)
        ident = const.tile([P, P], bf16)
        make_identity(nc, ident)

        # ---- resident weights ----
        # One dma_start per tensor: desc-gen (~0.7us/instr) is the scarce
        # resource, a single queue stripes packets over all 16 DMA engines.
        # sync queue carries A1's critical path (wq then x_q chunks);
        # scalar queue carries A2/A3 (wk, wv, x_k); gpsimd carries wo+mask.
        wq_sb = wpool.tile([P, DC, hs], bf16)
        wk_sb = wpool.tile([P, DC, hs], bf16)
        wv_sb = wpool.tile([P, DC, hs], bf16)
        wo_sb = wpool.tile([P, hpg // 2, d], bf16)
        nc.sync.dma_start(wk_sb[:], wkT_t.rearrange("c p h -> p c h"))

        # ---- resident activations ----
        rqT_sb = actp.tile([P, HC, q], bf16)                 # (hs, q)
        rkT_sb = actp.tile([P, HC, s], bf16)                 # (hs, s)
        rv_sb = actp.tile([P, SC, hpg * (adim + 1)], bf16)   # (s, hs + ones)
        nc.any.memset(rv_sb[:], 1.0)        # ones cols survive at 64::65

        mask_sb = [maskp.tile([P, SC, HALF], bf16, tag="mask",
                              name=f"mask{hf}") for hf in range(NH)]

        # ================= stage A =================
        with tc.tile_pool(name="xa", bufs=1) as xpool, \
             tc.tile_pool(name="xq", bufs=DC) as xqpool, \
             tc.tile_pool(name="psA", bufs=1, space="PSUM") as psA:
            # sync queue: (wq, x_q) interleaved per chunk so A1's c-loop
            # starts within a few us and streams just-in-time; then wo.
            # scalar queue: x_k per chunk (A2/A3), wv, then mask half 0.
            # gpsimd SWDGE is NOT used for DMA (~13 GB/s, far too slow).
            x_qs = [xqpool.tile([P, q], bf16, tag="xq", name=f"xq{c}")
                    for c in range(DC)]
            x_k = xpool.tile([P, DC, s], bf16, tag="xk")
            # A1's stream first: (wq, x_q) pairs; x_k (A2, needed ~30us
            # later) behind them; then wv/wo/mask0 (stage A3 / stage B).
            for c in range(DC):
                nc.sync.dma_start(wq_sb[:, c], wqT_t[c])
                nc.sync.dma_start(x_qs[c][:], qT_t[c])
            for c in range(DC):
                nc.sync.dma_start(x_k[:, c], kT_t[c])
            nc.sync.dma_start(wv_sb[:], wvT_t.rearrange("c p h -> p c h"))
            nc.sync.dma_start(wo_sb[:], wo[:])
            for sc in range(SC):
                nc.sync.dma_start(mask_sb[0][:, sc], maskT_t[:, sc, 0:HALF])

            # A1/A2: rqT and rkT. c outer, weight chunks stationary for
            # 4 streams, 8 psum accumulation groups live.
            for w_sb, x_res, dst in ((wq_sb, x_qs, rqT_sb),
                                     (wk_sb, x_k, rkT_sb)):
                ps8 = [psA.tile([P, 512], fp32, tag=f"pa{m * 4 + nq}",
                                name=f"psA_{m}_{nq}")
                       for m in range(HC) for nq in range(q // 512)]
                for c in range(DC):
                    x_c = x_res[c] if isinstance(x_res, list) else x_res[:, c]
                    for m in range(HC):
                        for nq in range(q // 512):
                            nc.tensor.matmul(
                                ps8[m * 4 + nq][:],
                                w_sb[:, c, m * P:(m + 1) * P],
                                x_c[:, nq * 512:(nq + 1) * 512],
                                start=(c == 0), stop=(c == DC - 1))
                for m in range(HC):
                    for nq in range(q // 512):
                        nc.scalar.copy(dst[:, m, nq * 512:(nq + 1) * 512],
                                       ps8[m * 4 + nq][:])

            # A3: rv (kT chunks stationary, wv moving)
            for sc in range(SC):
                ps = psA.tile([P, hs], fp32, tag=f"pa{sc % 2}", name="psA_rv")
                for c in range(DC):
                    nc.tensor.matmul(
                        ps[:], x_k[:, c, sc * P:(sc + 1) * P], wv_sb[:, c],
                        start=(c == 0), stop=(c == DC - 1))
                # scatter heads into 65-strided groups (col 64 stays 1.0)
                nc.scalar.copy(
                    rv_sb[:, sc].rearrange("p (h c) -> p h c",
                                           c=adim + 1)[:, :, 0:adim],
                    ps[:].rearrange("p (h c) -> p h c", c=adim))

        # ================= stage B =================
        with tc.tile_pool(name="psB", bufs=1, space="PSUM") as psB:
            tTs = {}
            av_ps = {}
            oT = {}

            def scores_step(half, h, sc):
                qlo = half * HALF
                hc, hp = h // 2, (h % 2) * adim
                ps = psB.tile([P, HALF], fp32, tag="sc", bufs=2,
                              name="ps_sc")
                for w in range(NW):
                    nc.tensor.matmul(
                        ps[:, w * 512:(w + 1) * 512],
                        rkT_sb[hp:hp + adim, hc, sc * P:(sc + 1) * P],
                        rqT_sb[hp:hp + adim, hc,
                               qlo + w * 512:qlo + (w + 1) * 512],
                        start=True, stop=False)
                # accumulate additive mask (+nbias) via identity matmul
                for w in range(NW):
                    nc.tensor.matmul(
                        ps[:, w * 512:(w + 1) * 512], ident[:],
                        mask_sb[half][:, sc, w * 512:(w + 1) * 512],
                        start=False, stop=True)
                # r = relu(scores + maskadd); t = r^2 (= relu(s')*s')
                r = rp.tile([P, HALF], bf16, tag="r", name="r_t")
                nc.scalar.activation(r[:], ps[:],
                                     mybir.ActivationFunctionType.Relu)
                tT = tTp.tile([P, HALF], bf16, tag="tT", name="tT_t")
                nc.vector.tensor_mul(tT[:], r[:], r[:])
                tTs[(half, h, sc)] = tT

            def av_step(half, h, sc):
                if sc == 0:
                    av_ps[h] = psB.tile([adim + 1, HALF], fp32, tag="av",
                                        bufs=2, name=f"av{h}")
                tT = tTs.pop((half, h, sc))
                for w in range(NW):
                    nc.tensor.matmul(
                        av_ps[h][:, w * 512:(w + 1) * 512],
                        rv_sb[:, sc, h * (adim + 1):(h + 1) * (adim + 1)],
                        tT[:, w * 512:(w + 1) * 512],
                        start=(sc == 0), stop=(sc == SC - 1))

            def scale_head_a(h):
                # den row (av psum row 64) -> bf16 SBUF, +eps guard
                av = av_ps[h]
                den = recp.tile([1, HALF], bf16, tag="den", name="den_t")
                nc.scalar.activation(den[:], av[adim:adim + 1, :],
                                     mybir.ActivationFunctionType.Copy,
                                     bias=1e-20)
                return den

            def scale_head_b(h, den):
                # broadcast den across 64 partitions via K=1 matmul,
                # then fast approximate reciprocal into SBUF fp32
                den2 = psB.tile([adim, HALF], fp32, tag="sc", bufs=2,
                                name="den2_t")
                for w in range(NW):
                    nc.tensor.matmul(den2[:, w * 512:(w + 1) * 512],
                                     ones64[:],
                                     den[:, w * 512:(w + 1) * 512],
                                     start=True, stop=True)
                rec2 = recp.tile([adim, HALF], fp32, tag="rec2",
                                 bufs=1, name="rec2_sb")
                nc.vector.reciprocal_approx_fast(rec2[:], den2[:])
                return rec2

            def scale_head_c(half, h, rec2):
                # even heads write pair partitions 0-63 directly; odd heads
                # go via a temp tile + SBUF->SBUF DMA into partitions 64-127
                # (compute engines cannot cross partitions, DMA can)
                av = av_ps.pop(h)
                pr = h // 2
                if h % 2 == 0:
                    o = oTp.tile([P, HALF], bf16, tag="oT", bufs=2,
                                 name=f"oT{half}_{pr}")
                    oT[(half, pr)] = o
                    nc.vector.tensor_mul(o[0:adim, :], av[0:adim, :], rec2[:])
                else:
                    tmp = oTp.tile([adim, HALF], bf16, tag="oTtmp", bufs=2,
                                   name="oTtmp")
                    nc.vector.tensor_mul(tmp[:], av[0:adim, :], rec2[:])
                    nc.sync.dma_start(oT[(half, pr)][adim:P, :], tmp[:])

            def outproj_qc(half, qc):
                # K=128: heads accumulated in pairs
                ps2 = psB.tile([P, d], fp32, tag="sc", bufs=2,
                               name=f"op{qc}")
                for pr in range(hpg // 2):
                    for dc in range(d // 512):
                        nc.tensor.matmul(
                            ps2[:, dc * 512:(dc + 1) * 512],
                            oT[(half, pr)][:, qc * P:(qc + 1) * P],
                            wo_sb[:, pr, dc * 512:(dc + 1) * 512],
                            start=(pr == 0), stop=(pr == hpg // 2 - 1))
                if qc == HALF // P - 1:
                    for pr in range(hpg // 2):
                        oT.pop((half, pr))
                ob = outp.tile([P, d], bf16, tag="ob", name="ob_t")
                nc.scalar.copy(ob[:], ps2[:])
                nc.sync.dma_start(out_t[half * (HALF // P) + qc], ob[:])

            # Uniform lag-2 software pipeline across head AND half
            # boundaries: scores at step i, AV at step i-2 — the PE stream
            # never pauses for a head's tail (that av flush happens under
            # the next head's first scores), keeping the HAM gate at 2.4GHz.
            # den/scale for head X runs 2-4 steps after its last AV; the
            # previous half's outproj runs as a blob early in the next half.
            STEPS = [(half, h, sc) for half in range(NH)
                     for h in range(hpg) for sc in range(SC)]
            N = len(STEPS)
            pend_a = {}   # step idx -> (half, h) whose den copy to emit
            pend_b = {}   # step idx -> (half, h, den) for broadcast+scale
            pend_op = {}  # step idx -> half whose outproj to emit
            for i in range(N + 2 * AV_LAG + 2):
                if i < N:
                    half, h, sc = STEPS[i]
                    if h == 0 and sc == 0 and half + 1 < NH:
                        for msc in range(SC):
                            nc.sync.dma_start(
                                mask_sb[half + 1][:, msc],
                                maskT_t[:, msc,
                                        (half + 1) * HALF:(half + 2) * HALF])
                    scores_step(half, h, sc)
                j = i - AV_LAG
                if 0 <= j < N:
                    jhalf, jh, jsc = STEPS[j]
                    av_step(jhalf, jh, jsc)
                    if jsc == SC - 1:
                        pend_a[i + 1] = (jhalf, jh)
                if i in pend_a:
                    ahalf, ah = pend_a.pop(i)
                    den = scale_head_a(ah)
                    pend_b[i + 2] = (ahalf, ah, den)
                if i in pend_b:
                    bhalf, bh, den = pend_b.pop(i)
                    rec2 = scale_head_b(bh, den)
                    scale_head_c(bhalf, bh, rec2)
                    if bh == hpg - 1:
                        pend_op[i + 1] = bhalf
                if i in pend_op:
                    for qc in range(HALF // P):
                        outproj_qc(pend_op[i], qc)
                    del pend_op[i]

    nc.compile()
    return nc


def _shard_inputs(iQ, iK, mask, Wq, Wkv, Wo, nbias):
    in_maps = []
    nb = np.float32(np.asarray(nbias).reshape(-1)[0])
    # additive mask: masked -> -1e30 (relu zeroes it), unmasked -> +nbias
    maskT_by_b = [np.ascontiguousarray(
        np.where(mask[b].T, np.float32(-1e30), nb)).astype(BF16)
        for b in range(B)]
    qT_by_b = [np.ascontiguousarray(iQ[b].T).astype(BF16) for b in range(B)]
    kT_by_b = [np.ascontiguousarray(iK[b].T).astype(BF16) for b in range(B)]
    scale = np.float32(1.0 / np.sqrt(np.float32(ADIM)))
    for ci in range(N_CORES):
        b, g = ci // GROUPS, ci % GROUPS
        hsl = slice(g * HS, (g + 1) * HS)
        # wo: [128, hpg//2, d]; pair p = heads (2p, 2p+1) stacked on the
        # partition dim
        wo_np = np.stack(
            [np.concatenate(
                [Wo[:, g * HS + h * ADIM:g * HS + (h + 1) * ADIM].T
                 for h in (2 * p, 2 * p + 1)], axis=0)
             for p in range(HPG // 2)], axis=1).astype(BF16)
        wo_np = np.ascontiguousarray(wo_np)
        in_maps.append({
            "qT": qT_by_b[b],
            "kT": kT_by_b[b],
            "wqT": np.ascontiguousarray((Wq[hsl] * scale).T).astype(BF16),
            "wkT": np.ascontiguousarray(Wkv[hsl].T).astype(BF16),
            "wvT": np.ascontiguousarray(
                Wkv[HSIZE + g * HS:HSIZE + (g + 1) * HS].T).astype(BF16),
            "wo": wo_np,
            "maskT": maskT_by_b[b],
        })
    return in_maps


def kernel(iQ, iK, mask, Wq, Wkv, Wo, nbias):
    global _COMPILED
    from concourse.bass_utils import run_bass_kernel_spmd

    if _COMPILED is None:
        _COMPILED = _build()
    in_maps = _shard_inputs(np.asarray(iQ, np.float32), np.asarray(iK, np.float32),
                            np.asarray(mask), np.asarray(Wq, np.float32),
                            np.asarray(Wkv, np.float32), np.asarray(Wo, np.float32),
                            np.asarray(nbias, np.float32))
    res = run_bass_kernel_spmd(_COMPILED, in_maps, list(range(N_CORES))).results
    out = np.zeros((B, Q, D), np.float32)
    for ci in range(N_CORES):
        out[ci // GROUPS] += np.asarray(res[ci]["out"], np.float32)
    return out


# revision 65
# speedup vs baseline: 1.0379x; 1.0238x over previous
"""Sparse cross-attention (squared-ReLU normalizer) on 8 TRN2 NeuronCores.

Sharding: 8 cores = batch(2) x head-group(4). Each core owns one batch and
4 of 16 heads (a 256-wide slice of hsize): Wq/Wkv column-parallel,
Wo row-parallel (bf16 partial outputs summed on host), mask replicated per
batch shard.

Per-core kernel (bf16 matmuls, fp32 PSUM). All DMAs ride the sync-engine
HWDGE queue (packets stripe over all 16 DMA engines; desc-gen on compute
engines would stall them), ordered so A1's inputs land first.

  Stage A: rqT (hs, q) / rkT (hs, s) with weight chunks stationary (reused
  across 4 N=512 streams, 8 live psum accumulation groups); rv
  (s, 4*(64+1)) with kT chunks stationary. 1/sqrt(adim) folded into Wq on
  the host.

  Stage B: one uniform lag-2 software pipeline over steps (half, h, sc)
  with q processed in 1024-halves — AV for step i-2 is emitted under step
  i's scores across head AND half boundaries, so the PE never idles and
  the HAM clock gate stays at 2.4 GHz:
    scores psum [128,1024] = rkT[h,sc]^T @ rqT[h]    (K=64, 2x N=512)
    psum += I_128 @ maskaddT chunk    (additive mask: masked -> -1e30,
      else +nbias, host-prepared; keeps the elementwise chain short)
    r = Relu(psum)  [ACT, bf16]; tT = r*r  [DVE]  (= relu(s')^2)
    av psum [65,1024] += rv[h,sc]^T @ tT   (rv stationary: 64 v cols +
      ones col -> row 64 = denominator)
  Head tail (pipelined 2-4 steps later): den row -> bf16 +eps [ACT],
  partition-broadcast via K=1 ones matmul, reciprocal_approx_fast [DVE],
  oT = av * rec2 [DVE]. Odd heads' oT moves to partitions 64-127 via
  SBUF->SBUF DMA so outproj runs K=128 on head PAIRS:
  out[qc, d] = sum_pr oTpair[pr][:, qc]^T @ wo_pair[pr], bf16 -> DMA.
  Each half's outproj is emitted as a blob a few steps into the next half
  (PE-dense region bridges the boundary).
"""

import numpy as np
import ml_dtypes

BF16 = ml_dtypes.bfloat16

B, Q, S, D = 2, 2048, 2048, 1024
NUM_HEAD, ADIM = 16, 64
HSIZE = NUM_HEAD * ADIM
N_CORES = 8
GROUPS = 4                  # head groups (tensor-parallel dim)
HPG = NUM_HEAD // GROUPS    # 4 heads per core
HS = HPG * ADIM             # 256: per-core hsize slice
IEPS = 1e-32
P = 128

_COMPILED = None


def _build(q=Q, s=S, d=D, hpg=HPG, adim=ADIM):
    """Build + compile the per-core Bass program. Returns the Bacc."""
    from contextlib import ExitStack
    import concourse.bass as bass
    import concourse.mybir as mybir
    import concourse.tile as tile
    from concourse import bacc
    from concourse.masks import make_identity

    fp32 = mybir.dt.float32
    bf16 = mybir.dt.bfloat16

    hs = hpg * adim          # 256
    DC = d // P              # 8 contraction chunks for projections
    SC = s // P              # 16 s chunks
    HC = hs // P             # 2 hsize-slice chunks
    HALF = 1024              # q processed in halves
    NH = q // HALF           # 2
    NW = HALF // 512         # 2 N=512 windows per half
    AV_LAG = 3

    nc = bacc.Bacc("TRN2", target_bir_lowering=False, debug=False,
                   num_devices=N_CORES)

    qT = nc.dram_tensor("qT", [d, q], bf16, kind="ExternalInput").ap()
    kT = nc.dram_tensor("kT", [d, s], bf16, kind="ExternalInput").ap()
    wqT = nc.dram_tensor("wqT", [d, hs], bf16, kind="ExternalInput").ap()
    wkT = nc.dram_tensor("wkT", [d, hs], bf16, kind="ExternalInput").ap()
    wvT = nc.dram_tensor("wvT", [d, hs], bf16, kind="ExternalInput").ap()
    # wo packed in head PAIRS: partitions 0-63 = head 2p, 64-127 = head 2p+1
    wo = nc.dram_tensor("wo", [P, hpg // 2, d], bf16,
                        kind="ExternalInput").ap()
    # additive mask: masked -> -1e30, unmasked -> nbias (host-prepared)
    maskT = nc.dram_tensor("maskT", [s, q], bf16, kind="ExternalInput").ap()
    out = nc.dram_tensor("out", [q, d], bf16, kind="ExternalOutput").ap()

    qT_t = qT.rearrange("(c p) q -> c p q", p=P)        # [DC, 128, q]
    kT_t = kT.rearrange("(c p) s -> c p s", p=P)
    wqT_t = wqT.rearrange("(c p) h -> c p h", p=P)      # [DC, 128, hs]
    wkT_t = wkT.rearrange("(c p) h -> c p h", p=P)
    wvT_t = wvT.rearrange("(c p) h -> c p h", p=P)
    maskT_t = maskT.rearrange("(c p) q -> p c q", p=P)  # [128, SC, q]
    out_t = out.rearrange("(c p) d -> c p d", p=P)      # [q/P, 128, d]

    with tile.TileContext(nc) as tc, ExitStack() as ctx:
        const = ctx.enter_context(tc.tile_pool(name="const", bufs=1))
        wpool = ctx.enter_context(tc.tile_pool(name="w", bufs=1))
        actp = ctx.enter_context(tc.tile_pool(name="act", bufs=1))
        maskp = ctx.enter_context(tc.tile_pool(name="mask", bufs=2))
        rp = ctx.enter_context(tc.tile_pool(name="r", bufs=2))
        tTp = ctx.enter_context(tc.tile_pool(name="tT", bufs=AV_LAG + 1))
        recp = ctx.enter_context(tc.tile_pool(name="rec", bufs=1))
        oTp = ctx.enter_context(tc.tile_pool(name="oT", bufs=hpg))
        outp = ctx.enter_context(tc.tile_pool(name="out", bufs=3))

        # ---- constants ----
        ones64 = const.tile([1, adim], bf16)
        nc.any.memset(ones64[:], 1.0)
        ident = const.tile([P, P], bf16)
        make_identity(nc, ident)

        # ---- resident weights ----
        # One dma_start per tensor: desc-gen (~0.7us/instr) is the scarce
        # resource, a single queue stripes packets over all 16 DMA engines.
        # sync queue carries A1's critical path (wq then x_q chunks);
        # scalar queue carries A2/A3 (wk, wv, x_k); gpsimd carries wo+mask.
        wq_sb = wpool.tile([P, DC, hs], bf16)
        wk_sb = wpool.tile([P, DC, hs], bf16)
        wv_sb = wpool.tile([P, DC, hs], bf16)
        wo_sb = wpool.tile([P, hpg // 2, d], bf16)
        nc.sync.dma_start(wk_sb[:], wkT_t.rearrange("c p h -> p c h"))

        # ---- resident activations ----
        rqT_sb = actp.tile([P, HC, q], bf16)                 # (hs, q)
        rkT_sb = actp.tile([P, HC, s], bf16)                 # (hs, s)
        rv_sb = actp.tile([P, SC, hpg * (adim + 1)], bf16)   # (s, hs + ones)
        nc.any.memset(rv_sb[:], 1.0)        # ones cols survive at 64::65

        mask_sb = [maskp.tile([P, SC, HALF], bf16, tag="mask",
                              name=f"mask{hf}") for hf in range(NH)]

        # ================= stage A =================
        with tc.tile_pool(name="xa", bufs=1) as xpool, \
             tc.tile_pool(name="xq", bufs=DC) as xqpool, \
             tc.tile_pool(name="psA", bufs=1, space="PSUM") as psA:
            # sync queue: (wq, x_q) interleaved per chunk so A1's c-loop
            # starts within a few us and streams just-in-time; then wo.
            # scalar queue: x_k per chunk (A2/A3), wv, then mask half 0.
            # gpsimd SWDGE is NOT used for DMA (~13 GB/s, far too slow).
            x_qs = [xqpool.tile([P, q], bf16, tag="xq", name=f"xq{c}")
                    for c in range(DC)]
            x_k = xpool.tile([P, DC, s], bf16, tag="xk")
            # A1's stream first: (wq, x_q) pairs; x_k (A2, needed ~30us
            # later) behind them; then wv/wo/mask0 (stage A3 / stage B).
            for c in range(DC):
                nc.sync.dma_start(wq_sb[:, c], wqT_t[c])
                nc.sync.dma_start(x_qs[c][:], qT_t[c])
            for c in range(DC):
                nc.sync.dma_start(x_k[:, c], kT_t[c])
            nc.sync.dma_start(wv_sb[:], wvT_t.rearrange("c p h -> p c h"))
            nc.sync.dma_start(wo_sb[:], wo[:])
            for sc in range(SC):
                nc.sync.dma_start(mask_sb[0][:, sc], maskT_t[:, sc, 0:HALF])

            # A1/A2: rqT and rkT. c outer, weight chunks stationary for
            # 4 streams, 8 psum accumulation groups live.
            for w_sb, x_res, dst in ((wq_sb, x_qs, rqT_sb),
                                     (wk_sb, x_k, rkT_sb)):
                ps8 = [psA.tile([P, 512], fp32, tag=f"pa{m * 4 + nq}",
                                name=f"psA_{m}_{nq}")
                       for m in range(HC) for nq in range(q // 512)]
                for c in range(DC):
                    x_c = x_res[c] if isinstance(x_res, list) else x_res[:, c]
                    for m in range(HC):
                        for nq in range(q // 512):
                            nc.tensor.matmul(
                                ps8[m * 4 + nq][:],
                                w_sb[:, c, m * P:(m + 1) * P],
                                x_c[:, nq * 512:(nq + 1) * 512],
                                start=(c == 0), stop=(c == DC - 1))
                for m in range(HC):
                    for nq in range(q // 512):
                        nc.scalar.copy(dst[:, m, nq * 512:(nq + 1) * 512],
                                       ps8[m * 4 + nq][:])

            # A3: rv (kT chunks stationary, wv moving)
            for sc in range(SC):
                ps = psA.tile([P, hs], fp32, tag=f"pa{sc % 2}", name="psA_rv")
                for c in range(DC):
                    nc.tensor.matmul(
                        ps[:], x_k[:, c, sc * P:(sc + 1) * P], wv_sb[:, c],
                        start=(c == 0), stop=(c == DC - 1))
                # scatter heads into 65-strided groups (col 64 stays 1.0)
                nc.scalar.copy(
                    rv_sb[:, sc].rearrange("p (h c) -> p h c",
                                           c=adim + 1)[:, :, 0:adim],
                    ps[:].rearrange("p (h c) -> p h c", c=adim))

        # ================= stage B =================
        with tc.tile_pool(name="psB", bufs=1, space="PSUM") as psB:
            tTs = {}
            av_ps = {}
            oT = {}

            def scores_step(half, h, sc):
                qlo = half * HALF
                hc, hp = h // 2, (h % 2) * adim
                ps = psB.tile([P, HALF], fp32, tag="sc", bufs=2,
                              name="ps_sc")
                for w in range(NW):
                    nc.tensor.matmul(
                        ps[:, w * 512:(w + 1) * 512],
                        rkT_sb[hp:hp + adim, hc, sc * P:(sc + 1) * P],
                        rqT_sb[hp:hp + adim, hc,
                               qlo + w * 512:qlo + (w + 1) * 512],
                        start=True, stop=False)
                # accumulate additive mask (+nbias) via identity matmul
                for w in range(NW):
                    nc.tensor.matmul(
                        ps[:, w * 512:(w + 1) * 512], ident[:],
                        mask_sb[half][:, sc, w * 512:(w + 1) * 512],
                        start=False, stop=True)
                # r = relu(scores + maskadd); t = r^2 (= relu(s')*s')
                r = rp.tile([P, HALF], bf16, tag="r", name="r_t")
                nc.scalar.activation(r[:], ps[:],
                                     mybir.ActivationFunctionType.Relu)
                tT = tTp.tile([P, HALF], bf16, tag="tT", name="tT_t")
                nc.vector.tensor_mul(tT[:], r[:], r[:])
                tTs[(half, h, sc)] = tT

            def av_step(half, h, sc):
                if sc == 0:
                    av_ps[h] = psB.tile([adim + 1, HALF], fp32, tag="av",
                                        bufs=2, name=f"av{h}")
                tT = tTs.pop((half, h, sc))
                for w in range(NW):
                    nc.tensor.matmul(
                        av_ps[h][:, w * 512:(w + 1) * 512],
                        rv_sb[:, sc, h * (adim + 1):(h + 1) * (adim + 1)],
                        tT[:, w * 512:(w + 1) * 512],
                        start=(sc == 0), stop=(sc == SC - 1))

            def scale_head_a(h):
                # den row (av psum row 64) -> bf16 SBUF, +eps guard
                av = av_ps[h]
                den = recp.tile([1, HALF], bf16, tag="den", name="den_t")
                nc.scalar.activation(den[:], av[adim:adim + 1, :],
                                     mybir.ActivationFunctionType.Copy,
                                     bias=1e-20)
                return den

            def scale_head_b(h, den):
                # broadcast den across 64 partitions via K=1 matmul,
                # then fast approximate reciprocal into SBUF fp32
                den2 = psB.tile([adim, HALF], fp32, tag="sc", bufs=2,
                                name="den2_t")
                for w in range(NW):
                    nc.tensor.matmul(den2[:, w * 512:(w + 1) * 512],
                                     ones64[:],
                                     den[:, w * 512:(w + 1) * 512],
                                     start=True, stop=True)
                rec2 = recp.tile([adim, HALF], fp32, tag="rec2",
                                 bufs=1, name="rec2_sb")
                nc.vector.reciprocal_approx_fast(rec2[:], den2[:])
                return rec2

            def scale_head_c(half, h, rec2):
                # even heads write pair partitions 0-63 directly; odd heads
                # go via a temp tile + SBUF->SBUF DMA into partitions 64-127
                # (compute engines cannot cross partitions, DMA can)
                av = av_ps.pop(h)
                pr = h // 2
                if h % 2 == 0:
                    o = oTp.tile([P, HALF], bf16, tag="oT", bufs=2,
                                 name=f"oT{half}_{pr}")
                    oT[(half, pr)] = o
                    nc.vector.tensor_mul(o[0:adim, :], av[0:adim, :], rec2[:])
                else:
                    tmp = oTp.tile([adim, HALF], bf16, tag="oTtmp", bufs=1,
                                   name="oTtmp")
                    nc.vector.tensor_mul(tmp[:], av[0:adim, :], rec2[:])
                    nc.sync.dma_start(oT[(half, pr)][adim:P, :], tmp[:])

            def outproj_qc(half, qc):
                # K=128: heads accumulated in pairs
                ps2 = psB.tile([P, d], fp32, tag="sc", bufs=2,
                               name=f"op{qc}")
                for pr in range(hpg // 2):
                    for dc in range(d // 512):
                        nc.tensor.matmul(
                            ps2[:, dc * 512:(dc + 1) * 512],
                            oT[(half, pr)][:, qc * P:(qc + 1) * P],
                            wo_sb[:, pr, dc * 512:(dc + 1) * 512],
                            start=(pr == 0), stop=(pr == hpg // 2 - 1))
                if qc == HALF // P - 1:
                    for pr in range(hpg // 2):
                        oT.pop((half, pr))
                ob = outp.tile([P, d], bf16, tag="ob", name="ob_t")
                nc.scalar.copy(ob[:], ps2[:])
                nc.sync.dma_start(out_t[half * (HALF // P) + qc], ob[:])

            # Uniform lag-2 software pipeline across head AND half
            # boundaries: scores at step i, AV at step i-2 — the PE stream
            # never pauses for a head's tail (that av flush happens under
            # the next head's first scores), keeping the HAM gate at 2.4GHz.
            # den/scale for head X runs 2-4 steps after its last AV; the
            # previous half's outproj runs as a blob early in the next half.
            STEPS = [(half, h, sc) for half in range(NH)
                     for h in range(hpg) for sc in range(SC)]
            N = len(STEPS)
            pend_a = {}   # step idx -> (half, h) whose den copy to emit
            pend_b = {}   # step idx -> (half, h, den) for broadcast+scale
            pend_op = {}  # step idx -> half whose outproj to emit
            for i in range(N + 2 * AV_LAG + 2):
                if i < N:
                    half, h, sc = STEPS[i]
                    if h == 0 and sc == 0 and half + 1 < NH:
                        for msc in range(SC):
                            nc.sync.dma_start(
                                mask_sb[half + 1][:, msc],
                                maskT_t[:, msc,
                                        (half + 1) * HALF:(half + 2) * HALF])
                    scores_step(half, h, sc)
                j = i - AV_LAG
                if 0 <= j < N:
                    jhalf, jh, jsc = STEPS[j]
                    av_step(jhalf, jh, jsc)
                    if jsc == SC - 1:
                        pend_a[i + 1] = (jhalf, jh)
                if i in pend_a:
                    ahalf, ah = pend_a.pop(i)
                    den = scale_head_a(ah)
                    pend_b[i + 2] = (ahalf, ah, den)
                if i in pend_b:
                    bhalf, bh, den = pend_b.pop(i)
                    rec2 = scale_head_b(bh, den)
                    scale_head_c(bhalf, bh, rec2)
                    if bh == hpg - 1:
                        pend_op[i + 1] = bhalf
                if i in pend_op:
                    for qc in range(HALF // P):
                        outproj_qc(pend_op[i], qc)
                    del pend_op[i]

    nc.compile()
    return nc


def _shard_inputs(iQ, iK, mask, Wq, Wkv, Wo, nbias):
    in_maps = []
    nb = np.float32(np.asarray(nbias).reshape(-1)[0])
    # additive mask: masked -> -1e30 (relu zeroes it), unmasked -> +nbias
    maskT_by_b = [np.ascontiguousarray(
        np.where(mask[b].T, np.float32(-1e30), nb)).astype(BF16)
        for b in range(B)]
    qT_by_b = [np.ascontiguousarray(iQ[b].T).astype(BF16) for b in range(B)]
    kT_by_b = [np.ascontiguousarray(iK[b].T).astype(BF16) for b in range(B)]
    scale = np.float32(1.0 / np.sqrt(np.float32(ADIM)))
    for ci in range(N_CORES):
        b, g = ci // GROUPS, ci % GROUPS
        hsl = slice(g * HS, (g + 1) * HS)
        # wo: [128, hpg//2, d]; pair p = heads (2p, 2p+1) stacked on the
        # partition dim
        wo_np = np.stack(
            [np.concatenate(
                [Wo[:, g * HS + h * ADIM:g * HS + (h + 1) * ADIM].T
                 for h in (2 * p, 2 * p + 1)], axis=0)
             for p in range(HPG // 2)], axis=1).astype(BF16)
        wo_np = np.ascontiguousarray(wo_np)
        in_maps.append({
            "qT": qT_by_b[b],
            "kT": kT_by_b[b],
            "wqT": np.ascontiguousarray((Wq[hsl] * scale).T).astype(BF16),
            "wkT": np.ascontiguousarray(Wkv[hsl].T).astype(BF16),
            "wvT": np.ascontiguousarray(
                Wkv[HSIZE + g * HS:HSIZE + (g + 1) * HS].T).astype(BF16),
            "wo": wo_np,
            "maskT": maskT_by_b[b],
        })
    return in_maps


def kernel(iQ, iK, mask, Wq, Wkv, Wo, nbias):
    global _COMPILED
    from concourse.bass_utils import run_bass_kernel_spmd

    if _COMPILED is None:
        _COMPILED = _build()
    in_maps = _shard_inputs(np.asarray(iQ, np.float32), np.asarray(iK, np.float32),
                            np.asarray(mask), np.asarray(Wq, np.float32),
                            np.asarray(Wkv, np.float32), np.asarray(Wo, np.float32),
                            np.asarray(nbias, np.float32))
    res = run_bass_kernel_spmd(_COMPILED, in_maps, list(range(N_CORES))).results
    out = np.zeros((B, Q, D), np.float32)
    for ci in range(N_CORES):
        out[ci // GROUPS] += np.asarray(res[ci]["out"], np.float32)
    return out


# revision 66
# speedup vs baseline: 1.0391x; 1.0012x over previous
"""Sparse cross-attention (squared-ReLU normalizer) on 8 TRN2 NeuronCores.

Sharding: 8 cores = batch(2) x head-group(4). Each core owns one batch and
4 of 16 heads (a 256-wide slice of hsize): Wq/Wkv column-parallel,
Wo row-parallel (bf16 partial outputs summed on host), mask replicated per
batch shard.

Per-core kernel (bf16 matmuls, fp32 PSUM). All DMAs ride the sync-engine
HWDGE queue (packets stripe over all 16 DMA engines; desc-gen on compute
engines would stall them), ordered so A1's inputs land first.

  Stage A: rqT (hs, q) / rkT (hs, s) with weight chunks stationary (reused
  across 4 N=512 streams, 8 live psum accumulation groups); rv
  (s, 4*(64+1)) with kT chunks stationary. 1/sqrt(adim) folded into Wq on
  the host.

  Stage B: one uniform lag-2 software pipeline over steps (half, h, sc)
  with q processed in 1024-halves — AV for step i-2 is emitted under step
  i's scores across head AND half boundaries, so the PE never idles and
  the HAM clock gate stays at 2.4 GHz:
    scores psum [128,1024] = rkT[h,sc]^T @ rqT[h]    (K=64, 2x N=512)
    psum += I_128 @ maskaddT chunk    (additive mask: masked -> -1e30,
      else +nbias, host-prepared; keeps the elementwise chain short)
    r = Relu(psum)  [ACT, bf16]; tT = r*r  [DVE]  (= relu(s')^2)
    av psum [65,1024] += rv[h,sc]^T @ tT   (rv stationary: 64 v cols +
      ones col -> row 64 = denominator)
  Head tail (pipelined 2-4 steps later): den row -> bf16 +eps [ACT],
  partition-broadcast via K=1 ones matmul, reciprocal_approx_fast [DVE],
  oT = av * rec2 [DVE]. Odd heads' oT moves to partitions 64-127 via
  SBUF->SBUF DMA so outproj runs K=128 on head PAIRS:
  out[qc, d] = sum_pr oTpair[pr][:, qc]^T @ wo_pair[pr], bf16 -> DMA.
  Each half's outproj is emitted as a blob a few steps into the next half
  (PE-dense region bridges the boundary).
"""

import numpy as np
import ml_dtypes

BF16 = ml_dtypes.bfloat16

B, Q, S, D = 2, 2048, 2048, 1024
NUM_HEAD, ADIM = 16, 64
HSIZE = NUM_HEAD * ADIM
N_CORES = 8
GROUPS = 4                  # head groups (tensor-parallel dim)
HPG = NUM_HEAD // GROUPS    # 4 heads per core
HS = HPG * ADIM             # 256: per-core hsize slice
IEPS = 1e-32
P = 128

_COMPILED = None


def _build(q=Q, s=S, d=D, hpg=HPG, adim=ADIM):
    """Build + compile the per-core Bass program. Returns the Bacc."""
    from contextlib import ExitStack
    import concourse.bass as bass
    import concourse.mybir as mybir
    import concourse.tile as tile
    from concourse import bacc
    from concourse.masks import make_identity

    fp32 = mybir.dt.float32
    bf16 = mybir.dt.bfloat16

    hs = hpg * adim          # 256
    DC = d // P              # 8 contraction chunks for projections
    SC = s // P              # 16 s chunks
    HC = hs // P             # 2 hsize-slice chunks
    HALF = 1024              # q processed in halves
    NH = q // HALF           # 2
    NW = HALF // 512         # 2 N=512 windows per half
    AV_LAG = 3

    nc = bacc.Bacc("TRN2", target_bir_lowering=False, debug=False,
                   num_devices=N_CORES)

    qT = nc.dram_tensor("qT", [d, q], bf16, kind="ExternalInput").ap()
    kT = nc.dram_tensor("kT", [d, s], bf16, kind="ExternalInput").ap()
    wqT = nc.dram_tensor("wqT", [d, hs], bf16, kind="ExternalInput").ap()
    wkT = nc.dram_tensor("wkT", [d, hs], bf16, kind="ExternalInput").ap()
    wvT = nc.dram_tensor("wvT", [d, hs], bf16, kind="ExternalInput").ap()
    # wo packed in head PAIRS: partitions 0-63 = head 2p, 64-127 = head 2p+1
    wo = nc.dram_tensor("wo", [P, hpg // 2, d], bf16,
                        kind="ExternalInput").ap()
    # additive mask: masked -> -1e30, unmasked -> nbias (host-prepared)
    maskT = nc.dram_tensor("maskT", [s, q], bf16, kind="ExternalInput").ap()
    out = nc.dram_tensor("out", [q, d], bf16, kind="ExternalOutput").ap()

    qT_t = qT.rearrange("(c p) q -> c p q", p=P)        # [DC, 128, q]
    kT_t = kT.rearrange("(c p) s -> c p s", p=P)
    wqT_t = wqT.rearrange("(c p) h -> c p h", p=P)      # [DC, 128, hs]
    wkT_t = wkT.rearrange("(c p) h -> c p h", p=P)
    wvT_t = wvT.rearrange("(c p) h -> c p h", p=P)
    maskT_t = maskT.rearrange("(c p) q -> p c q", p=P)  # [128, SC, q]
    out_t = out.rearrange("(c p) d -> c p d", p=P)      # [q/P, 128, d]

    with tile.TileContext(nc) as tc, ExitStack() as ctx:
        const = ctx.enter_context(tc.tile_pool(name="const", bufs=1))
        wpool = ctx.enter_context(tc.tile_pool(name="w", bufs=1))
        actp = ctx.enter_context(tc.tile_pool(name="act", bufs=1))
        maskp = ctx.enter_context(tc.tile_pool(name="mask", bufs=2))
        rp = ctx.enter_context(tc.tile_pool(name="r", bufs=2))
        tTp = ctx.enter_context(tc.tile_pool(name="tT", bufs=AV_LAG + 1))
        recp = ctx.enter_context(tc.tile_pool(name="rec", bufs=1))
        oTp = ctx.enter_context(tc.tile_pool(name="oT", bufs=hpg))
        outp = ctx.enter_context(tc.tile_pool(name="out", bufs=3))

        # ---- constants ----
        ones64 = const.tile([1, adim], bf16)
        nc.any.memset(ones64[:], 1.0)
        ident = const.tile([P, P], bf16)
        make_identity(nc, ident)

        # ---- resident weights ----
        # One dma_start per tensor: desc-gen (~0.7us/instr) is the scarce
        # resource, a single queue stripes packets over all 16 DMA engines.
        # sync queue carries A1's critical path (wq then x_q chunks);
        # scalar queue carries A2/A3 (wk, wv, x_k); gpsimd carries wo+mask.
        wq_sb = wpool.tile([P, DC, hs], bf16)
        wk_sb = wpool.tile([P, DC, hs], bf16)
        wv_sb = wpool.tile([P, DC, hs], bf16)
        wo_sb = wpool.tile([P, hpg // 2, d], bf16)
        nc.sync.dma_start(wk_sb[:], wkT_t.rearrange("c p h -> p c h"))

        # ---- resident activations ----
        rqT_sb = actp.tile([P, HC, q], bf16)                 # (hs, q)
        rkT_sb = actp.tile([P, HC, s], bf16)                 # (hs, s)
        rv_sb = actp.tile([P, SC, hpg * (adim + 1)], bf16)   # (s, hs + ones)
        nc.any.memset(rv_sb[:], 1.0)        # ones cols survive at 64::65

        mask_sb = [maskp.tile([P, SC, HALF], bf16, tag="mask",
                              name=f"mask{hf}") for hf in range(NH)]

        # ================= stage A =================
        with tc.tile_pool(name="xa", bufs=1) as xpool, \
             tc.tile_pool(name="xq", bufs=DC) as xqpool, \
             tc.tile_pool(name="psA", bufs=1, space="PSUM") as psA:
            # sync queue: (wq, x_q) interleaved per chunk so A1's c-loop
            # starts within a few us and streams just-in-time; then wo.
            # scalar queue: x_k per chunk (A2/A3), wv, then mask half 0.
            # gpsimd SWDGE is NOT used for DMA (~13 GB/s, far too slow).
            x_qs = [xqpool.tile([P, q], bf16, tag="xq", name=f"xq{c}")
                    for c in range(DC)]
            x_k = xpool.tile([P, DC, s], bf16, tag="xk")
            # A1's stream first: (wq, x_q) pairs; x_k (A2, needed ~30us
            # later) behind them; then wv/wo/mask0 (stage A3 / stage B).
            for c in range(DC):
                nc.sync.dma_start(wq_sb[:, c], wqT_t[c])
                nc.sync.dma_start(x_qs[c][:], qT_t[c])
            for c in range(DC):
                nc.sync.dma_start(x_k[:, c], kT_t[c])
            nc.sync.dma_start(wv_sb[:], wvT_t.rearrange("c p h -> p c h"))
            nc.sync.dma_start(wo_sb[:], wo[:])
            for sc in range(SC):
                nc.sync.dma_start(mask_sb[0][:, sc], maskT_t[:, sc, 0:HALF])

            # A1/A2: rqT and rkT. c outer, weight chunks stationary for
            # 4 streams, 8 psum accumulation groups live.
            for w_sb, x_res, dst in ((wq_sb, x_qs, rqT_sb),
                                     (wk_sb, x_k, rkT_sb)):
                ps8 = [psA.tile([P, 512], fp32, tag=f"pa{m * 4 + nq}",
                                name=f"psA_{m}_{nq}")
                       for m in range(HC) for nq in range(q // 512)]
                for c in range(DC):
                    x_c = x_res[c] if isinstance(x_res, list) else x_res[:, c]
                    for m in range(HC):
                        for nq in range(q // 512):
                            nc.tensor.matmul(
                                ps8[m * 4 + nq][:],
                                w_sb[:, c, m * P:(m + 1) * P],
                                x_c[:, nq * 512:(nq + 1) * 512],
                                start=(c == 0), stop=(c == DC - 1))
                for m in range(HC):
                    for nq in range(q // 512):
                        nc.scalar.copy(dst[:, m, nq * 512:(nq + 1) * 512],
                                       ps8[m * 4 + nq][:])

            # A3: rv (kT chunks stationary, wv moving)
            for sc in range(SC):
                ps = psA.tile([P, hs], fp32, tag=f"pa{sc % 2}", name="psA_rv")
                for c in range(DC):
                    nc.tensor.matmul(
                        ps[:], x_k[:, c, sc * P:(sc + 1) * P], wv_sb[:, c],
                        start=(c == 0), stop=(c == DC - 1))
                # scatter heads into 65-strided groups (col 64 stays 1.0)
                nc.scalar.copy(
                    rv_sb[:, sc].rearrange("p (h c) -> p h c",
                                           c=adim + 1)[:, :, 0:adim],
                    ps[:].rearrange("p (h c) -> p h c", c=adim))

        # ================= stage B =================
        with tc.tile_pool(name="psB", bufs=1, space="PSUM") as psB:
            tTs = {}
            av_ps = {}
            oT = {}

            def scores_step(half, h, sc):
                qlo = half * HALF
                hc, hp = h // 2, (h % 2) * adim
                ps = psB.tile([P, HALF], fp32, tag="sc", bufs=2,
                              name="ps_sc")
                for w in range(NW):
                    nc.tensor.matmul(
                        ps[:, w * 512:(w + 1) * 512],
                        rkT_sb[hp:hp + adim, hc, sc * P:(sc + 1) * P],
                        rqT_sb[hp:hp + adim, hc,
                               qlo + w * 512:qlo + (w + 1) * 512],
                        start=True, stop=False)
                # accumulate additive mask (+nbias) via identity matmul
                for w in range(NW):
                    nc.tensor.matmul(
                        ps[:, w * 512:(w + 1) * 512], ident[:],
                        mask_sb[half][:, sc, w * 512:(w + 1) * 512],
                        start=False, stop=True)
                # r = relu(scores + maskadd); t = r^2 (= relu(s')*s')
                r = rp.tile([P, HALF], bf16, tag="r", name="r_t")
                nc.scalar.activation(r[:], ps[:],
                                     mybir.ActivationFunctionType.Relu)
                tT = tTp.tile([P, HALF], bf16, tag="tT", name="tT_t")
                nc.vector.tensor_mul(tT[:], r[:], r[:])
                tTs[(half, h, sc)] = tT

            def av_step(half, h, sc):
                if sc == 0:
                    av_ps[h] = psB.tile([adim + 1, HALF], fp32, tag="av",
                                        bufs=2, name=f"av{h}")
                tT = tTs.pop((half, h, sc))
                for w in range(NW):
                    nc.tensor.matmul(
                        av_ps[h][:, w * 512:(w + 1) * 512],
                        rv_sb[:, sc, h * (adim + 1):(h + 1) * (adim + 1)],
                        tT[:, w * 512:(w + 1) * 512],
                        start=(sc == 0), stop=(sc == SC - 1))

            def scale_head_a(h):
                # den row (av psum row 64) -> bf16 SBUF, +eps guard
                av = av_ps[h]
                den = recp.tile([1, HALF], bf16, tag="den", name="den_t")
                nc.scalar.activation(den[:], av[adim:adim + 1, :],
                                     mybir.ActivationFunctionType.Copy,
                                     bias=1e-20)
                return den

            def scale_head_b(h, den):
                # broadcast den across 64 partitions via K=1 matmul,
                # then fast approximate reciprocal into SBUF fp32
                den2 = psB.tile([adim, HALF], fp32, tag="sc", bufs=2,
                                name="den2_t")
                for w in range(NW):
                    nc.tensor.matmul(den2[:, w * 512:(w + 1) * 512],
                                     ones64[:],
                                     den[:, w * 512:(w + 1) * 512],
                                     start=True, stop=True)
                rec2 = recp.tile([adim, HALF], fp32, tag="rec2",
                                 bufs=1, name="rec2_sb")
                nc.vector.reciprocal_approx_fast(rec2[:], den2[:])
                return rec2

            def scale_head_c(half, h, rec2):
                # even heads write pair partitions 0-63 directly; odd heads
                # go via a temp tile + SBUF->SBUF DMA into partitions 64-127
                # (compute engines cannot cross partitions, DMA can)
                av = av_ps.pop(h)
                pr = h // 2
                if h % 2 == 0:
                    o = oTp.tile([P, HALF], bf16, tag="oT", bufs=2,
                                 name=f"oT{half}_{pr}")
                    oT[(half, pr)] = o
                    nc.vector.tensor_mul(o[0:adim, :], av[0:adim, :], rec2[:])
                else:
                    tmp = oTp.tile([adim, HALF], bf16, tag="oTtmp", bufs=1,
                                   name="oTtmp")
                    nc.vector.tensor_mul(tmp[:], av[0:adim, :], rec2[:])
                    nc.sync.dma_start(oT[(half, pr)][adim:P, :], tmp[:])

            def outproj_qc(half, qc):
                # K=128: heads accumulated in pairs
                ps2 = psB.tile([P, d], fp32, tag="sc", bufs=2,
                               name=f"op{qc}")
                for pr in range(hpg // 2):
                    for dc in range(d // 512):
                        nc.tensor.matmul(
                            ps2[:, dc * 512:(dc + 1) * 512],
                            oT[(half, pr)][:, qc * P:(qc + 1) * P],
                            wo_sb[:, pr, dc * 512:(dc + 1) * 512],
                            start=(pr == 0), stop=(pr == hpg // 2 - 1))
                if qc == HALF // P - 1:
                    for pr in range(hpg // 2):
                        oT.pop((half, pr))
                # split evict across ACT+DVE and DMA per half-row so the
                # blob paces at PE rate, not the serial evict+DMA rate
                ob = outp.tile([P, d], bf16, tag="ob", name="ob_t")
                hd = d // 2
                nc.scalar.copy(ob[:, 0:hd], ps2[:, 0:hd])
                nc.vector.tensor_copy(ob[:, hd:], ps2[:, hd:])
                orow = out_t[half * (HALF // P) + qc]
                nc.sync.dma_start(orow[:, 0:hd], ob[:, 0:hd])
                nc.sync.dma_start(orow[:, hd:], ob[:, hd:])

            # Uniform lag-2 software pipeline across head AND half
            # boundaries: scores at step i, AV at step i-2 — the PE stream
            # never pauses for a head's tail (that av flush happens under
            # the next head's first scores), keeping the HAM gate at 2.4GHz.
            # den/scale for head X runs 2-4 steps after its last AV; the
            # previous half's outproj runs as a blob early in the next half.
            STEPS = [(half, h, sc) for half in range(NH)
                     for h in range(hpg) for sc in range(SC)]
            N = len(STEPS)
            pend_a = {}   # step idx -> (half, h) whose den copy to emit
            pend_b = {}   # step idx -> (half, h, den) for broadcast+scale
            pend_op = {}  # step idx -> half whose outproj to emit
            for i in range(N + 2 * AV_LAG + 2):
                if i < N:
                    half, h, sc = STEPS[i]
                    if h == 0 and sc == 0 and half + 1 < NH:
                        for msc in range(SC):
                            nc.sync.dma_start(
                                mask_sb[half + 1][:, msc],
                                maskT_t[:, msc,
                                        (half + 1) * HALF:(half + 2) * HALF])
                    scores_step(half, h, sc)
                j = i - AV_LAG
                if 0 <= j < N:
                    jhalf, jh, jsc = STEPS[j]
                    av_step(jhalf, jh, jsc)
                    if jsc == SC - 1:
                        pend_a[i + 1] = (jhalf, jh)
                if i in pend_a:
                    ahalf, ah = pend_a.pop(i)
                    den = scale_head_a(ah)
                    pend_b[i + 2] = (ahalf, ah, den)
                if i in pend_b:
                    bhalf, bh, den = pend_b.pop(i)
                    rec2 = scale_head_b(bh, den)
                    scale_head_c(bhalf, bh, rec2)
                    if bh == hpg - 1:
                        pend_op[i + 1] = bhalf
                if i in pend_op:
                    for qc in range(HALF // P):
                        outproj_qc(pend_op[i], qc)
                    del pend_op[i]

    nc.compile()
    return nc


def _shard_inputs(iQ, iK, mask, Wq, Wkv, Wo, nbias):
    in_maps = []
    nb = np.float32(np.asarray(nbias).reshape(-1)[0])
    # additive mask: masked -> -1e30 (relu zeroes it), unmasked -> +nbias
    maskT_by_b = [np.ascontiguousarray(
        np.where(mask[b].T, np.float32(-1e30), nb)).astype(BF16)
        for b in range(B)]
    qT_by_b = [np.ascontiguousarray(iQ[b].T).astype(BF16) for b in range(B)]
    kT_by_b = [np.ascontiguousarray(iK[b].T).astype(BF16) for b in range(B)]
    scale = np.float32(1.0 / np.sqrt(np.float32(ADIM)))
    for ci in range(N_CORES):
        b, g = ci // GROUPS, ci % GROUPS
        hsl = slice(g * HS, (g + 1) * HS)
        # wo: [128, hpg//2, d]; pair p = heads (2p, 2p+1) stacked on the
        # partition dim
        wo_np = np.stack(
            [np.concatenate(
                [Wo[:, g * HS + h * ADIM:g * HS + (h + 1) * ADIM].T
                 for h in (2 * p, 2 * p + 1)], axis=0)
             for p in range(HPG // 2)], axis=1).astype(BF16)
        wo_np = np.ascontiguousarray(wo_np)
        in_maps.append({
            "qT": qT_by_b[b],
            "kT": kT_by_b[b],
            "wqT": np.ascontiguousarray((Wq[hsl] * scale).T).astype(BF16),
            "wkT": np.ascontiguousarray(Wkv[hsl].T).astype(BF16),
            "wvT": np.ascontiguousarray(
                Wkv[HSIZE + g * HS:HSIZE + (g + 1) * HS].T).astype(BF16),
            "wo": wo_np,
            "maskT": maskT_by_b[b],
        })
    return in_maps


def kernel(iQ, iK, mask, Wq, Wkv, Wo, nbias):
    global _COMPILED
    from concourse.bass_utils import run_bass_kernel_spmd

    if _COMPILED is None:
        _COMPILED = _build()
    in_maps = _shard_inputs(np.asarray(iQ, np.float32), np.asarray(iK, np.float32),
                            np.asarray(mask), np.asarray(Wq, np.float32),
                            np.asarray(Wkv, np.float32), np.asarray(Wo, np.float32),
                            np.asarray(nbias, np.float32))
    res = run_bass_kernel_spmd(_COMPILED, in_maps, list(range(N_CORES))).results
    out = np.zeros((B, Q, D), np.float32)
    for ci in range(N_CORES):
        out[ci // GROUPS] += np.asarray(res[ci]["out"], np.float32)
    return out
